# revision 58
# baseline (speedup 1.0000x reference)
"""Trainium2 Bass kernel: batched multi-head cross-attention.

Reference computation (per batch element b):
    q = x @ Wq; k,v = split(context @ Wkv)
    per head: attn = softmax(q k^T / 8); o = attn @ v
    out = concat_heads(o) @ Wo + bo

Sharding: pure data parallel — batch B=8, one batch element per NeuronCore,
no collectives. Inside each core we use a fully "transposed" dataflow so no
on-device transposes are ever needed:

    QT[d,i]  = Wq^T  @ xT          (lhsT=Wq  natural, rhs=x^T fed from host)
    KT[d,j]  = Wk^T  @ cT
    V[j,d]   = cT^T  @ Wv          (lhsT=cT chunk,    rhs=Wv)
    ST[j,i]  = KT_h^T @ QT_h       (per head, contraction d=64)
    PT       = exp(ST / 8)          (no max-subtraction: scores are O(6) so
                                     exp is safe in fp32, and softmax is
                                     shift-invariant => exact same result)
    OunT,l   = [V_h | 1]^T @ PT     (ones column fused into V gives the
                                     softmax denominator row for free)
    OnT      = OunT * (1/l)         (1/l broadcast across partitions via a
                                     partition-step-0 SBUF->SBUF DMA)
    outT     = Wo^T @ OnT + bo

Host feeds x^T / context^T and bf16 weight copies per core; host transposes
outT back after gather. Matmul inputs are bf16 (full-rate PE), accumulation
is fp32 in PSUM.
"""

import numpy as np
import ml_dtypes

B, N, M, D = 8, 1024, 1024, 512
H, DH = 8, 64
KC = 4          # 512 contraction -> 4 chunks of 128
IC = 2          # 1024 free dim -> 2 chunks of 512
JC = 8          # 1024 keys -> 8 chunks of 128
N_CORES = 8

_BF16 = ml_dtypes.bfloat16
_CACHE = {}
LAST_RUN = None  # BassKernelResults of the most recent launch (for test.py)


def _build_nc():
    import concourse.bass as bass
    import concourse.mybir as mybir
    import concourse.tile as tile
    from concourse import bacc

    f32 = mybir.dt.float32
    bf16 = mybir.dt.bfloat16
    Exp = mybir.ActivationFunctionType.Exp

    # Bacc (not raw Bass): its compile() pass redistributes semaphore waits
    # (hardware allows only 1 wait per instruction, 2 on event semaphores).
    nc = bacc.Bacc()

    xt = nc.declare_dram_parameter("xt", [D, N], bf16, isOutput=False)
    ct = nc.declare_dram_parameter("ct", [D, M], bf16, isOutput=False)
    # Host-provided zeros: DMA'd into the kt zero-halves over the idle
    # SWDGE queue, keeping Act/DVE free of ~6us of memsets in the head.
    zb = nc.declare_dram_parameter("zb", [128, N], bf16, isOutput=False)
    wq = nc.declare_dram_parameter("wq", [D, D], bf16, isOutput=False)
    wk = nc.declare_dram_parameter("wk", [D, D], bf16, isOutput=False)
    wv = nc.declare_dram_parameter("wv", [D, D], bf16, isOutput=False)
    wo = nc.declare_dram_parameter("wo", [D, D], bf16, isOutput=False)
    bo = nc.declare_dram_parameter("bo", [D, 1], f32, isOutput=False)
    # Output in bf16 (host upcasts): halves the 2MB output drain, which
    # gates the kernel tail; costs ~0.3% extra rel err (budget is 2e-2).
    outT = nc.declare_dram_parameter("outT", [D, N], bf16, isOutput=True)

    with tile.TileContext(nc) as tc:
        with (
            tc.tile_pool(name="singles", bufs=1) as singles,
            tc.tile_pool(name="pt", bufs=6) as ptp,
            tc.tile_pool(name="pout", bufs=8) as poutp,
        ):
            # ---- phase A: loads, interleaved across both HWDGE queues so the
            # first projection matmuls can start after ~2 transfers ----
            def sb(shape, dt, tag):
                return singles.tile(shape, dt, tag=tag, name=tag)

            wq_sb = [sb([128, D], bf16, f"wq{c}") for c in range(KC)]
            xt_sb = [sb([128, N], bf16, f"xt{c}") for c in range(KC)]
            # Weights/ct arrive as half- or whole-tensor DMAs (fewer
            # ~0.66us trigger instructions on the queue engines); chunk c
            # lives at [:, c, :].
            ct_big = sb([128, KC, M], bf16, "ct")
            wk_big = sb([128, KC, D], bf16, "wk")
            wv_big = sb([128, KC, D], bf16, "wv")
            wo_big = sb([128, KC, D], bf16, "wo")
            ct_sb = [ct_big[:, c, :] for c in range(KC)]
            wk_sb = [wk_big[:, c, :] for c in range(KC)]
            wv_sb = [wv_big[:, c, :] for c in range(KC)]
            wo_sb = [wo_big[:, c, :] for c in range(KC)]

            # PE warm-up during the DMA load phase: HAM (the PE clock gate)
            # un-throttles 1.2->2.4GHz only after ~3.4us of sustained
            # full-array activity, and re-throttles after an idle window.
            # Dummy full-width matmuls on an SBUF tile that depends on
            # nothing let the PE ramp while loads stream.  The warm memset
            # must be DVE's FIRST instruction (before the padding memsets
            # below) and the count small: the PE queue is in-order, so
            # excess warmups head-of-line block the real projections.
            warm_sb = sb([128, 512], bf16, "warm")
            nc.vector.memset(warm_sb, 0.0)
            with tc.tile_pool(name="pwarm", bufs=1, space="PSUM") as pwarm:
                pswm = pwarm.tile([128, 512], f32, tag="pwarm", name="pwarm")
                for _ in range(10):
                    nc.tensor.matmul(
                        pswm, lhsT=warm_sb[:, 0:128], rhs=warm_sb,
                        start=True, stop=True,
                    )

            # Loads interleaved across the two HWDGE queues (7 triggers
            # each; the ~0.66us trigger instrs are themselves a serial
            # cost), both queues carrying every tensor so each lands at
            # half-tensor latency, ordered by consumption: Q waves eat
            # xt/wq first, then K needs wk+ct, V needs wv.  The late-needed
            # wo/bo and the kt zero-halves ride the slow-but-idle SWDGE.
            wkr = wk[:, :].rearrange("(h c p) d -> p h c d", p=128, h=2)
            wvr = wv[:, :].rearrange("(h c p) d -> p h c d", p=128, h=2)
            ctr = ct[:, :].rearrange("(h c p) m -> p h c m", p=128, h=2)
            wkb = wk_big.rearrange("p (h c) d -> p h c d", h=2)
            wvb = wv_big.rearrange("p (h c) d -> p h c d", h=2)
            ctb = ct_big.rearrange("p (h c) m -> p h c m", h=2)
            # xt/wq stream first (Q starts on them ~10us in), then wk+ct
            # for K, then wv; the warm-hold bridge between the Q casts and
            # the K chains below covers the ~3us ct-arrival shortfall.
            nc.sync.dma_start(out=wq_sb[0], in_=wq[0:128, :])
            nc.scalar.dma_start(out=xt_sb[0], in_=xt[0:128, :])
            nc.sync.dma_start(out=xt_sb[1], in_=xt[128:256, :])
            nc.scalar.dma_start(out=wq_sb[1], in_=wq[128:256, :])
            nc.sync.dma_start(out=wq_sb[2], in_=wq[256:384, :])
            nc.scalar.dma_start(out=xt_sb[2], in_=xt[256:384, :])
            nc.sync.dma_start(out=xt_sb[3], in_=xt[384:512, :])
            nc.scalar.dma_start(out=wq_sb[3], in_=wq[384:512, :])
            nc.sync.dma_start(out=wkb[:, 0], in_=wkr[:, 0])
            nc.scalar.dma_start(out=wkb[:, 1], in_=wkr[:, 1])
            # ct chunk 3 rides SWDGE (first in its queue): slow per-byte but
            # it starts at boot, landing well before the HWDGE stream gets
            # there — shrinking the ct-arrival hole before the K chains.
            nc.gpsimd.dma_start(out=ct_sb[3], in_=ct[384:512, :])
            nc.sync.dma_start(out=ctb[:, 0], in_=ctr[:, 0])
            nc.scalar.dma_start(out=ct_sb[2], in_=ct[256:384, :])
            nc.sync.dma_start(out=wvb[:, 0], in_=wvr[:, 0])
            nc.scalar.dma_start(out=wvb[:, 1], in_=wvr[:, 1])
            bo_sb = sb([128, KC, 1], f32, "bo")

            qt_sb = [sb([128, N], bf16, f"qt{c}") for c in range(KC)]
            # K^T kept in two zero-padded copies so the QK^T stationary can be
            # a full [128,128] tile: HAM throttles the PE clock to 1.2GHz when
            # matmuls light up only part of the array (DH=64 -> 64 stationary
            # rows), which is why the baseline's whole attention phase ran at
            # half clock.  kt0 holds even heads in partitions 0-63 (64-127
            # zero); kt1 holds odd heads in partitions 64-127 (0-63 zero).
            # The zero rows multiply the other head's Q rows into nothing, so
            # the result is exact while the array reads as fully active.
            kt0_sb = [sb([128, M], bf16, f"kt0_{c}") for c in range(KC)]
            kt1_sb = [sb([128, M], bf16, f"kt1_{c}") for c in range(KC)]
            # V padded from 65 to 128 columns ([V | ones | zeros]) for the
            # same reason: 65 stationary columns -> half the array -> K=4/8.
            v_sb = [sb([128, H, 128], bf16, f"v{j}") for j in range(JC)]
            # kt zero-halves come from the host zeros buffer over SWDGE
            # (before wo/bo in its queue — these gate the first QK^T):
            # zeroing them on Act or DVE costs ~6us of exactly the engine
            # time that gates the projection PSUM drains.
            for c in range(KC):
                nc.gpsimd.dma_start(out=kt0_sb[c][64:128, :], in_=zb[0:64, :])
                nc.gpsimd.dma_start(out=kt1_sb[c][0:64, :], in_=zb[64:128, :])
            nc.gpsimd.dma_start(
                out=wo_big, in_=wo[:, :].rearrange("(c p) d -> p c d", p=128)
            )
            nc.gpsimd.dma_start(
                out=bo_sb, in_=bo[:, :].rearrange("(c p) o -> p c o", p=128)
            )
            # V ones-column and zero-pad: no deps, emitted up front while
            # DVE is otherwise idle (its phase-B work is only the kt1
            # casts now); the V-copies later fill cols 0..DH-1.
            for j in range(JC):
                nc.vector.memset(v_sb[j][:, :, DH:DH + 1], 1.0)
                nc.vector.memset(v_sb[j][:, :, DH + 1:], 0.0)
            o_sb = [sb([128, N], bf16, f"o{c}") for c in range(KC)]
            on_sb = [sb([128, N], bf16, f"on{c}") for c in range(KC)]
            r_sb = [sb([128, N], f32, f"r{c}") for c in range(2)]
            # Per-head softmax-denominator rows, staged at partition 0
            # (compute engines may only address partition bases 0/32/64/96).
            lstage = [sb([1, N], f32, f"lst{h}") for h in range(H)]
            linv_st = [sb([1, N], f32, f"linv{h}") for h in range(H)]

            # ---- phase B: projections ----
            # Wave order: all 8 PSUM tiles of a projection accumulate chunk
            # kc together, so each arriving wq/xt/ct chunk immediately
            # releases 8 matmuls — the PE never idle-waits a serial
            # accumulation chain on a half-loaded input (idle would also
            # HAM-re-throttle the clock).
            with tc.tile_pool(name="pp", bufs=8, space="PSUM") as pp:
                # Q-proj in kc-waves (it's paced by the xt/wq chunks still
                # streaming in), with casts alternating Act/DVE in dc order
                # so neither engine's drain serializes the pool rotation and
                # the dc0 tiles attention needs first land first.
                ps = {}
                for dc in range(KC):
                    for ic in range(IC):
                        ps[dc, ic] = pp.tile(
                            [128, 512], f32, tag="pp", name="pp"
                        )
                for kc in range(KC):
                    for dc in range(KC):
                        for ic in range(IC):
                            nc.tensor.matmul(
                                ps[dc, ic],
                                lhsT=wq_sb[kc][:, dc * 128:(dc + 1) * 128],
                                rhs=xt_sb[kc][:, ic * 512:(ic + 1) * 512],
                                start=(kc == 0),
                                stop=(kc == KC - 1),
                            )
                for dc in range(KC):
                    for ic in range(IC):
                        sl = slice(ic * 512, (ic + 1) * 512)
                        if ic == 0:
                            nc.scalar.copy(qt_sb[dc][:, sl], ps[dc, ic])
                        else:
                            nc.vector.tensor_copy(
                                qt_sb[dc][:, sl], ps[dc, ic]
                            )
                # Warm-hold bridge over the ct-arrival gap: a PE hole here
                # would HAM-re-throttle the clock for the rest of the head.
                br = pp.tile([128, 512], f32, tag="pp", name="br")
                for _ in range(12):
                    nc.tensor.matmul(
                        br, lhsT=warm_sb[:, 0:128], rhs=warm_sb,
                        start=True, stop=True,
                    )
                # K-proj tile-major: all of ct+wk is resident by now, so
                # each (dc,ic) tile's 4-matmul chain completes ~0.9us apart
                # and its casts (kt0 on Act, kt1 on DVE) pipeline instead of
                # bursting after a final wave.
                for dc in range(KC):
                    for ic in range(IC):
                        kp = pp.tile([128, 512], f32, tag="pp", name="kp")
                        for kc in range(KC):
                            nc.tensor.matmul(
                                kp,
                                lhsT=wk_sb[kc][:, dc * 128:(dc + 1) * 128],
                                rhs=ct_sb[kc][:, ic * 512:(ic + 1) * 512],
                                start=(kc == 0),
                                stop=(kc == KC - 1),
                            )
                        sl = slice(ic * 512, (ic + 1) * 512)
                        nc.scalar.copy(kt0_sb[dc][0:64, sl], kp[0:64, :])
                        nc.vector.tensor_copy(
                            kt1_sb[dc][64:128, sl], kp[64:128, :]
                        )
                # V-proj jc-major for the same reason.
                for jc in range(JC):
                    vp = pp.tile([128, 512], f32, tag="pp", name="vp")
                    for kc in range(KC):
                        nc.tensor.matmul(
                            vp,
                            lhsT=ct_sb[kc][:, jc * 128:(jc + 1) * 128],
                            rhs=wv_sb[kc],
                            start=(kc == 0),
                            stop=(kc == KC - 1),
                        )
                    nc.vector.tensor_copy(
                        v_sb[jc][:, :, 0:DH],
                        vp.rearrange("p (h d) -> p h d", h=H),
                    )

            # ---- phase C: attention head by head; normalization prep is
            # emitted incrementally per head pair so the kernel tail is short.
            # The QK^T for step s+1 is emitted BEFORE the PV for step s: the
            # PE queue is in-order, so without the prefetch PV(s) head-of-line
            # blocks the queue on exp(s) (~250ns stall per step); with it the
            # PE always has an independent QK^T to chew while Act catches up.
            with (
                tc.tile_pool(name="psS", bufs=2, space="PSUM") as psS,
                tc.tile_pool(name="psO", bufs=2, space="PSUM") as psO,
            ):
                steps = [(h, jc) for h in range(H) for jc in range(JC)]
                pso_t = {}
                pending = None  # (h, jc, ptile) whose PV is not yet emitted

                def emit_qk_exp(h, jc):
                    dc = h // 2
                    kt_h = (kt0_sb, kt1_sb)[h % 2][dc]
                    pss = psS.tile([128, N], f32, tag="psS", name="psS")
                    for ic in range(IC):
                        nc.tensor.matmul(
                            pss[:, ic * 512:(ic + 1) * 512],
                            lhsT=kt_h[:, jc * 128:(jc + 1) * 128],
                            rhs=qt_sb[dc][:, ic * 512:(ic + 1) * 512],
                            start=True,
                            stop=True,
                        )
                    ptile = ptp.tile([128, N], bf16, tag="pt", name="pt")
                    nc.scalar.activation(out=ptile, in_=pss, func=Exp, scale=0.125)
                    return ptile

                def emit_pv(h, jc, ptile):
                    if jc == 0:
                        pso_t[h] = psO.tile([128, N], f32, tag="psO", name="psO")
                    for ic in range(IC):
                        nc.tensor.matmul(
                            pso_t[h][:, ic * 512:(ic + 1) * 512],
                            lhsT=v_sb[jc][:, h, :],
                            rhs=ptile[:, ic * 512:(ic + 1) * 512],
                            start=(jc == 0),
                            stop=(jc == JC - 1),
                        )

                def emit_norm(h):
                    dc = h // 2
                    pb = (h % 2) * 64
                    pso = pso_t.pop(h)
                    nc.vector.tensor_copy(lstage[h], pso[DH:DH + 1, :])
                    if h == H - 1:
                        # Last head: finish the PSUM reads before the recip
                        # so the psO bank frees for the out-projection ASAP.
                        nc.vector.tensor_copy(
                            o_sb[h // 2][64:128, :], pso[0:DH, :]
                        )
                    nc.vector.reciprocal_approx_fast(
                        out=linv_st[h], in_=lstage[h]
                    )
                    nc.gpsimd.partition_broadcast(
                        r_sb[h % 2], linv_st[h][0:1, :]
                    )
                    # Even heads: the normalize mul reads O^T straight out
                    # of PSUM (no staging copy — saves ~1.2us DVE per head).
                    # Odd heads need a partition shift 0:64 -> 64:128, which
                    # only plain copies are known to support, so they keep
                    # the staged form.  Split by ic-half so the ic=0
                    # out-proj groups unblock as soon as the first half
                    # lands.
                    if pb == 0:
                        for icm in range(IC):
                            nc.vector.tensor_mul(
                                on_sb[dc][0:64, icm * 512:(icm + 1) * 512],
                                pso[0:DH, icm * 512:(icm + 1) * 512],
                                r_sb[h % 2][0:64, icm * 512:(icm + 1) * 512],
                            )
                    else:
                        if h != H - 1:
                            nc.vector.tensor_copy(
                                o_sb[dc][64:128, :], pso[0:DH, :]
                            )
                        for icm in range(IC):
                            nc.vector.tensor_mul(
                                on_sb[dc][64:128, icm * 512:(icm + 1) * 512],
                                o_sb[dc][64:128, icm * 512:(icm + 1) * 512],
                                r_sb[h % 2][64:128, icm * 512:(icm + 1) * 512],
                            )

                for h, jc in steps:
                    ptile = emit_qk_exp(h, jc)
                    if pending is not None:
                        ph, pjc, pt_ = pending
                        emit_pv(ph, pjc, pt_)
                        if pjc == JC - 1:
                            emit_norm(ph)
                    pending = (h, jc, ptile)
                ph, pjc, pt_ = pending
                emit_pv(ph, pjc, pt_)
                emit_norm(ph)
                # Warm-hold: dependency-free matmuls bridge the PE idle gap
                # between the last attention matmul and the out-projection
                # (whose PSUM banks free only after the norm chain drains
                # through DVE).  Without them HAM sees ~4us of idle and
                # halves the PE clock for the whole out-projection.
                bridge = psS.tile([128, N], f32, tag="psS", name="bridge")
                for _ in range(10):
                    nc.tensor.matmul(
                        bridge[:, 0:512], lhsT=warm_sb[:, 0:128], rhs=warm_sb,
                        start=True, stop=True,
                    )

            # ---- phase E: output projection + bias.  All eight tiles
            # bulk-accumulate hc=0..2 before any hc=3 stop-matmul so the
            # tail after the last head's normalize chain is just 8
            # stop-matmuls + bias + stores.
            with tc.tile_pool(name="pf", bufs=8, space="PSUM") as pf:
                tiles = [pf.tile([128, 512], f32, tag="pf", name=f"pf{q}")
                         for q in range(8)]
                # Per-tile grouped accumulation: a tile whose PSUM bank is
                # still held by the attention pools delays only its own
                # matmuls, not (via the in-order PE queue) everyone else's.
                for q, psf in enumerate(tiles):
                    ec, ic = q // IC, q % IC
                    for hc in range(KC - 1):
                        nc.tensor.matmul(
                            psf,
                            lhsT=wo_sb[hc][:, ec * 128:(ec + 1) * 128],
                            rhs=on_sb[hc][:, ic * 512:(ic + 1) * 512],
                            start=(hc == 0), stop=False,
                        )
                ot_ec = [singles.tile([128, N], bf16, tag=f"ot{e}",
                                      name=f"ot{e}") for e in range(KC)]
                for q, psf in enumerate(tiles):
                    ec, ic = q // IC, q % IC
                    nc.tensor.matmul(
                        psf,
                        lhsT=wo_sb[3][:, ec * 128:(ec + 1) * 128],
                        rhs=on_sb[3][:, ic * 512:(ic + 1) * 512],
                        start=False, stop=True,
                    )
                    # Alternate Act (idle at the tail) and DVE for the bias
                    # adds so neither engine serializes the drain.
                    dst = ot_ec[ec][:, ic * 512:(ic + 1) * 512]
                    if q % 2 == 0:
                        nc.scalar.add(dst, psf, bo_sb[:, ec, :])
                    else:
                        nc.vector.tensor_scalar_add(dst, psf, bo_sb[:, ec, :])
                    if ic == IC - 1:
                        # One [128,1024] store per ec row-block: half the
                        # DMA triggers, on the two HWDGE queues.
                        store_q = (nc.sync, nc.scalar)[ec % 2]
                        store_q.dma_start(
                            out=outT[ec * 128:(ec + 1) * 128, :],
                            in_=ot_ec[ec],
                        )
    # Runs Bacc's compile passes (register allocation, wait splitting) —
    # run_bass_via_pjrt does not finalize for us.
    nc.finalize()
    return nc


def _ensure_ntff_hook():
    """Install antenv.axon_hooks if the image lacks it, registering the
    ctypes NTFF-profile hook against libaxon_pjrt.so. Without this,
    run_bass_kernel_spmd(trace=True)/BASS_TRACE=1 crashes on import."""
    import contextlib
    import ctypes
    import os
    import sys
    import types

    try:
        import antenv.axon_hooks  # noqa: F401
        return
    except ImportError:
        pass
    try:
        import antenv
    except ImportError:
        return

    state = {"hook": None}
    mod = types.ModuleType("antenv.axon_hooks")
    mod.set_axon_ntff_profile_hook = lambda h: state.__setitem__("hook", h)
    mod.get_axon_ntff_profile_hook = lambda: state["hook"]
    sys.modules["antenv.axon_hooks"] = mod
    antenv.axon_hooks = mod

    so_path = "/opt/axon/libaxon_pjrt.so"
    if not os.path.exists(so_path):
        return
    try:
        lib = ctypes.CDLL(so_path)
    except OSError:
        return
    if not hasattr(lib, "axon_start_nrt_profile"):
        return
    lib.axon_start_nrt_profile.argtypes = [
        ctypes.POINTER(ctypes.c_int64), ctypes.c_size_t,
    ]
    lib.axon_start_nrt_profile.restype = ctypes.c_int64
    lib.axon_stop_nrt_profile.argtypes = [ctypes.c_char_p]
    lib.axon_stop_nrt_profile.restype = ctypes.c_int64

    @contextlib.contextmanager
    def _hook(output_dir, device_ids):
        import jax
        jax.devices()  # force PJRT init so the .so's client exists
        if device_ids:
            ids = (ctypes.c_int64 * len(device_ids))(*device_ids)
            rc = lib.axon_start_nrt_profile(ids, len(device_ids))
        else:
            rc = lib.axon_start_nrt_profile(None, 0)
        if rc != 0:
            raise RuntimeError(f"axon_start_nrt_profile rc={rc}")
        try:
            yield
        finally:
            n = lib.axon_stop_nrt_profile(str(output_dir).encode())
            if n <= 0:
                print(f"ntff profile: rc={n} (no profile output)")

    state["hook"] = _hook


def kernel(x, context, Wq, Wkv, Wo, bo):
    global LAST_RUN
    _ensure_ntff_hook()
    from concourse import bass_utils

    if "nc" not in _CACHE:
        _CACHE["nc"] = _build_nc()
    nc = _CACHE["nc"]

    wq = np.ascontiguousarray(Wq, dtype=np.float32).astype(_BF16)
    wk = np.ascontiguousarray(Wkv[:, :D], dtype=np.float32).astype(_BF16)
    wv = np.ascontiguousarray(Wkv[:, D:], dtype=np.float32).astype(_BF16)
    wo = np.ascontiguousarray(Wo, dtype=np.float32).astype(_BF16)
    bo_ = np.ascontiguousarray(np.asarray(bo, dtype=np.float32).reshape(D, 1))
    zb = np.zeros((128, N), dtype=_BF16)

    in_maps = []
    for b in range(B):
        in_maps.append({
            "xt": np.ascontiguousarray(np.asarray(x[b], np.float32).T).astype(_BF16),
            "ct": np.ascontiguousarray(np.asarray(context[b], np.float32).T).astype(_BF16),
            "wq": wq, "wk": wk, "wv": wv, "wo": wo,
            "bo": bo_, "zb": zb,
        })

    LAST_RUN = bass_utils.run_bass_kernel_spmd(nc, in_maps, list(range(N_CORES)))
    out = np.empty((B, N, D), dtype=np.float32)
    for b in range(B):
        out[b] = LAST_RUN.results[b]["outT"].astype(np.float32).T
    return out



# revision 62
# speedup vs baseline: 1.0136x; 1.0136x over previous
"""Trainium2 Bass kernel: batched multi-head cross-attention.

Reference computation (per batch element b):
    q = x @ Wq; k,v = split(context @ Wkv)
    per head: attn = softmax(q k^T / 8); o = attn @ v
    out = concat_heads(o) @ Wo + bo

Sharding: pure data parallel — batch B=8, one batch element per NeuronCore,
no collectives. Inside each core we use a fully "transposed" dataflow so no
on-device transposes are ever needed:

    QT[d,i]  = Wq^T  @ xT          (lhsT=Wq  natural, rhs=x^T fed from host)
    KT[d,j]  = Wk^T  @ cT
    V[j,d]   = cT^T  @ Wv          (lhsT=cT chunk,    rhs=Wv)
    ST[j,i]  = KT_h^T @ QT_h       (per head, contraction d=64)
    PT       = exp(ST / 8)          (no max-subtraction: scores are O(6) so
                                     exp is safe in fp32, and softmax is
                                     shift-invariant => exact same result)
    OunT,l   = [V_h | 1]^T @ PT     (ones column fused into V gives the
                                     softmax denominator row for free)
    OnT      = OunT * (1/l)         (1/l broadcast across partitions via a
                                     partition-step-0 SBUF->SBUF DMA)
    outT     = Wo^T @ OnT + bo

Host feeds x^T / context^T and bf16 weight copies per core; host transposes
outT back after gather. Matmul inputs are bf16 (full-rate PE), accumulation
is fp32 in PSUM.
"""

import numpy as np
import ml_dtypes

B, N, M, D = 8, 1024, 1024, 512
H, DH = 8, 64
KC = 4          # 512 contraction -> 4 chunks of 128
IC = 2          # 1024 free dim -> 2 chunks of 512
JC = 8          # 1024 keys -> 8 chunks of 128
N_CORES = 8

_BF16 = ml_dtypes.bfloat16
_CACHE = {}
LAST_RUN = None  # BassKernelResults of the most recent launch (for test.py)


def _build_nc():
    import concourse.bass as bass
    import concourse.mybir as mybir
    import concourse.tile as tile
    from concourse import bacc

    f32 = mybir.dt.float32
    bf16 = mybir.dt.bfloat16
    Exp = mybir.ActivationFunctionType.Exp

    # Bacc (not raw Bass): its compile() pass redistributes semaphore waits
    # (hardware allows only 1 wait per instruction, 2 on event semaphores).
    nc = bacc.Bacc()

    xt = nc.declare_dram_parameter("xt", [D, N], bf16, isOutput=False)
    ct = nc.declare_dram_parameter("ct", [D, M], bf16, isOutput=False)
    # Host-provided zeros: DMA'd into the kt zero-halves over the idle
    # SWDGE queue, keeping Act/DVE free of ~6us of memsets in the head.
    zb = nc.declare_dram_parameter("zb", [128, N], bf16, isOutput=False)
    wq = nc.declare_dram_parameter("wq", [D, D], bf16, isOutput=False)
    wk = nc.declare_dram_parameter("wk", [D, D], bf16, isOutput=False)
    wv = nc.declare_dram_parameter("wv", [D, D], bf16, isOutput=False)
    wo = nc.declare_dram_parameter("wo", [D, D], bf16, isOutput=False)
    bo = nc.declare_dram_parameter("bo", [D, 1], f32, isOutput=False)
    # Output in bf16 (host upcasts): halves the 2MB output drain, which
    # gates the kernel tail; costs ~0.3% extra rel err (budget is 2e-2).
    outT = nc.declare_dram_parameter("outT", [D, N], bf16, isOutput=True)

    with tile.TileContext(nc) as tc:
        with (
            tc.tile_pool(name="singles", bufs=1) as singles,
            tc.tile_pool(name="pt", bufs=6) as ptp,
            tc.tile_pool(name="pout", bufs=8) as poutp,
        ):
            # ---- phase A: loads, interleaved across both HWDGE queues so the
            # first projection matmuls can start after ~2 transfers ----
            def sb(shape, dt, tag):
                return singles.tile(shape, dt, tag=tag, name=tag)

            wq_sb = [sb([128, D], bf16, f"wq{c}") for c in range(KC)]
            xt_sb = [sb([128, N], bf16, f"xt{c}") for c in range(KC)]
            # Weights/ct arrive as half- or whole-tensor DMAs (fewer
            # ~0.66us trigger instructions on the queue engines); chunk c
            # lives at [:, c, :].
            ct_big = sb([128, KC, M], bf16, "ct")
            wk_big = sb([128, KC, D], bf16, "wk")
            wv_big = sb([128, KC, D], bf16, "wv")
            wo_big = sb([128, KC, D], bf16, "wo")
            ct_sb = [ct_big[:, c, :] for c in range(KC)]
            wk_sb = [wk_big[:, c, :] for c in range(KC)]
            wv_sb = [wv_big[:, c, :] for c in range(KC)]
            wo_sb = [wo_big[:, c, :] for c in range(KC)]

            # PE warm-up during the DMA load phase: HAM (the PE clock gate)
            # un-throttles 1.2->2.4GHz only after ~3.4us of sustained
            # full-array activity, and re-throttles after an idle window.
            # Dummy full-width matmuls on an SBUF tile that depends on
            # nothing let the PE ramp while loads stream.  The warm memset
            # must be DVE's FIRST instruction (before the padding memsets
            # below) and the count small: the PE queue is in-order, so
            # excess warmups head-of-line block the real projections.
            warm_sb = sb([128, 512], bf16, "warm")
            nc.vector.memset(warm_sb, 0.0)
            with tc.tile_pool(name="pwarm", bufs=1, space="PSUM") as pwarm:
                pswm = pwarm.tile([128, 512], f32, tag="pwarm", name="pwarm")
                for _ in range(10):
                    nc.tensor.matmul(
                        pswm, lhsT=warm_sb[:, 0:128], rhs=warm_sb,
                        start=True, stop=True,
                    )

            # Loads interleaved across the two HWDGE queues (7 triggers
            # each; the ~0.66us trigger instrs are themselves a serial
            # cost), both queues carrying every tensor so each lands at
            # half-tensor latency, ordered by consumption: Q waves eat
            # xt/wq first, then K needs wk+ct, V needs wv.  The late-needed
            # wo/bo and the kt zero-halves ride the slow-but-idle SWDGE.
            wkr = wk[:, :].rearrange("(h c p) d -> p h c d", p=128, h=2)
            wvr = wv[:, :].rearrange("(h c p) d -> p h c d", p=128, h=2)
            ctr = ct[:, :].rearrange("(h c p) m -> p h c m", p=128, h=2)
            wkb = wk_big.rearrange("p (h c) d -> p h c d", h=2)
            wvb = wv_big.rearrange("p (h c) d -> p h c d", h=2)
            ctb = ct_big.rearrange("p (h c) m -> p h c m", h=2)
            # xt/wq stream first (Q starts on them ~10us in), then wk+ct
            # for K, then wv; the warm-hold bridge between the Q casts and
            # the K chains below covers the ~3us ct-arrival shortfall.
            nc.sync.dma_start(out=wq_sb[0], in_=wq[0:128, :])
            nc.scalar.dma_start(out=xt_sb[0], in_=xt[0:128, :])
            nc.sync.dma_start(out=xt_sb[1], in_=xt[128:256, :])
            nc.scalar.dma_start(out=wq_sb[1], in_=wq[128:256, :])
            nc.sync.dma_start(out=wq_sb[2], in_=wq[256:384, :])
            nc.scalar.dma_start(out=xt_sb[2], in_=xt[256:384, :])
            nc.sync.dma_start(out=xt_sb[3], in_=xt[384:512, :])
            nc.scalar.dma_start(out=wq_sb[3], in_=wq[384:512, :])
            nc.sync.dma_start(out=wkb[:, 0], in_=wkr[:, 0])
            nc.scalar.dma_start(out=wkb[:, 1], in_=wkr[:, 1])
            nc.sync.dma_start(out=ctb[:, 0], in_=ctr[:, 0])
            nc.scalar.dma_start(out=ctb[:, 1], in_=ctr[:, 1])
            nc.sync.dma_start(out=wvb[:, 0], in_=wvr[:, 0])
            nc.scalar.dma_start(out=wvb[:, 1], in_=wvr[:, 1])
            bo_sb = sb([128, KC, 1], f32, "bo")

            qt_sb = [sb([128, N], bf16, f"qt{c}") for c in range(KC)]
            # K^T kept in two zero-padded copies so the QK^T stationary can be
            # a full [128,128] tile: HAM throttles the PE clock to 1.2GHz when
            # matmuls light up only part of the array (DH=64 -> 64 stationary
            # rows), which is why the baseline's whole attention phase ran at
            # half clock.  kt0 holds even heads in partitions 0-63 (64-127
            # zero); kt1 holds odd heads in partitions 64-127 (0-63 zero).
            # The zero rows multiply the other head's Q rows into nothing, so
            # the result is exact while the array reads as fully active.
            kt0_sb = [sb([128, M], bf16, f"kt0_{c}") for c in range(KC)]
            kt1_sb = [sb([128, M], bf16, f"kt1_{c}") for c in range(KC)]
            # V padded from 65 to 128 columns ([V | ones | zeros]) for the
            # same reason: 65 stationary columns -> half the array -> K=4/8.
            v_sb = [sb([128, H, 128], bf16, f"v{j}") for j in range(JC)]
            # kt zero-halves come from the host zeros buffer over SWDGE
            # (before wo/bo in its queue — these gate the first QK^T):
            # zeroing them on Act or DVE costs ~6us of exactly the engine
            # time that gates the projection PSUM drains.
            for c in range(KC):
                nc.gpsimd.dma_start(out=kt0_sb[c][64:128, :], in_=zb[0:64, :])
                nc.gpsimd.dma_start(out=kt1_sb[c][0:64, :], in_=zb[64:128, :])
            nc.gpsimd.dma_start(
                out=wo_big, in_=wo[:, :].rearrange("(c p) d -> p c d", p=128)
            )
            nc.gpsimd.dma_start(
                out=bo_sb, in_=bo[:, :].rearrange("(c p) o -> p c o", p=128)
            )
            # V ones-column and zero-pad: no deps, emitted up front while
            # DVE is otherwise idle (its phase-B work is only the kt1
            # casts now); the V-copies later fill cols 0..DH-1.
            for j in range(JC):
                nc.vector.memset(v_sb[j][:, :, DH:DH + 1], 1.0)
                nc.vector.memset(v_sb[j][:, :, DH + 1:], 0.0)
            o_sb = [sb([128, N], bf16, f"o{c}") for c in range(KC)]
            on_sb = [sb([128, N], bf16, f"on{c}") for c in range(KC)]
            r_sb = [sb([128, N], f32, f"r{c}") for c in range(2)]
            # Per-head softmax-denominator rows, staged at partition 0
            # (compute engines may only address partition bases 0/32/64/96).
            lstage = [sb([1, N], f32, f"lst{h}") for h in range(H)]
            linv_st = [sb([1, N], f32, f"linv{h}") for h in range(H)]

            # ---- phase B: projections ----
            # Wave order: all 8 PSUM tiles of a projection accumulate chunk
            # kc together, so each arriving wq/xt/ct chunk immediately
            # releases 8 matmuls — the PE never idle-waits a serial
            # accumulation chain on a half-loaded input (idle would also
            # HAM-re-throttle the clock).
            with tc.tile_pool(name="pp", bufs=8, space="PSUM") as pp:
                # Q-proj in kc-waves (it's paced by the xt/wq chunks still
                # streaming in), with casts alternating Act/DVE in dc order
                # so neither engine's drain serializes the pool rotation and
                # the dc0 tiles attention needs first land first.
                ps = {}
                for dc in range(KC):
                    for ic in range(IC):
                        ps[dc, ic] = pp.tile(
                            [128, 512], f32, tag="pp", name="pp"
                        )
                for kc in range(KC):
                    for dc in range(KC):
                        for ic in range(IC):
                            nc.tensor.matmul(
                                ps[dc, ic],
                                lhsT=wq_sb[kc][:, dc * 128:(dc + 1) * 128],
                                rhs=xt_sb[kc][:, ic * 512:(ic + 1) * 512],
                                start=(kc == 0),
                                stop=(kc == KC - 1),
                            )
                for dc in range(KC):
                    for ic in range(IC):
                        sl = slice(ic * 512, (ic + 1) * 512)
                        if ic == 0:
                            nc.scalar.copy(qt_sb[dc][:, sl], ps[dc, ic])
                        else:
                            nc.vector.tensor_copy(
                                qt_sb[dc][:, sl], ps[dc, ic]
                            )
                # Warm-hold bridge over the ct-arrival gap: a PE hole here
                # would HAM-re-throttle the clock for the rest of the head.
                br = pp.tile([128, 512], f32, tag="pp", name="br")
                for _ in range(12):
                    nc.tensor.matmul(
                        br, lhsT=warm_sb[:, 0:128], rhs=warm_sb,
                        start=True, stop=True,
                    )
                # K-proj tile-major: all of ct+wk is resident by now, so
                # each (dc,ic) tile's 4-matmul chain completes ~0.9us apart
                # and its casts (kt0 on Act, kt1 on DVE) pipeline instead of
                # bursting after a final wave.
                for dc in range(KC):
                    for ic in range(IC):
                        kp = pp.tile([128, 512], f32, tag="pp", name="kp")
                        for kc in range(KC):
                            nc.tensor.matmul(
                                kp,
                                lhsT=wk_sb[kc][:, dc * 128:(dc + 1) * 128],
                                rhs=ct_sb[kc][:, ic * 512:(ic + 1) * 512],
                                start=(kc == 0),
                                stop=(kc == KC - 1),
                            )
                        sl = slice(ic * 512, (ic + 1) * 512)
                        nc.scalar.copy(kt0_sb[dc][0:64, sl], kp[0:64, :])
                        nc.vector.tensor_copy(
                            kt1_sb[dc][64:128, sl], kp[64:128, :]
                        )
                # V-proj jc-major for the same reason.
                for jc in range(JC):
                    vp = pp.tile([128, 512], f32, tag="pp", name="vp")
                    for kc in range(KC):
                        nc.tensor.matmul(
                            vp,
                            lhsT=ct_sb[kc][:, jc * 128:(jc + 1) * 128],
                            rhs=wv_sb[kc],
                            start=(kc == 0),
                            stop=(kc == KC - 1),
                        )
                    nc.vector.tensor_copy(
                        v_sb[jc][:, :, 0:DH],
                        vp.rearrange("p (h d) -> p h d", h=H),
                    )

            # ---- phase C: attention head by head; normalization prep is
            # emitted incrementally per head pair so the kernel tail is short.
            # The QK^T for step s+1 is emitted BEFORE the PV for step s: the
            # PE queue is in-order, so without the prefetch PV(s) head-of-line
            # blocks the queue on exp(s) (~250ns stall per step); with it the
            # PE always has an independent QK^T to chew while Act catches up.
            with (
                tc.tile_pool(name="psS", bufs=2, space="PSUM") as psS,
                tc.tile_pool(name="psO", bufs=2, space="PSUM") as psO,
            ):
                steps = [(h, jc) for h in range(H) for jc in range(JC)]
                pso_t = {}
                pending = None  # (h, jc, ptile) whose PV is not yet emitted

                def emit_qk_exp(h, jc):
                    dc = h // 2
                    kt_h = (kt0_sb, kt1_sb)[h % 2][dc]
                    pss = psS.tile([128, N], f32, tag="psS", name="psS")
                    for ic in range(IC):
                        nc.tensor.matmul(
                            pss[:, ic * 512:(ic + 1) * 512],
                            lhsT=kt_h[:, jc * 128:(jc + 1) * 128],
                            rhs=qt_sb[dc][:, ic * 512:(ic + 1) * 512],
                            start=True,
                            stop=True,
                        )
                    ptile = ptp.tile([128, N], bf16, tag="pt", name="pt")
                    nc.scalar.activation(out=ptile, in_=pss, func=Exp, scale=0.125)
                    return ptile

                def emit_pv(h, jc, ptile):
                    if jc == 0:
                        pso_t[h] = psO.tile([128, N], f32, tag="psO", name="psO")
                    for ic in range(IC):
                        nc.tensor.matmul(
                            pso_t[h][:, ic * 512:(ic + 1) * 512],
                            lhsT=v_sb[jc][:, h, :],
                            rhs=ptile[:, ic * 512:(ic + 1) * 512],
                            start=(jc == 0),
                            stop=(jc == JC - 1),
                        )

                def emit_norm(h):
                    dc = h // 2
                    pb = (h % 2) * 64
                    pso = pso_t.pop(h)
                    nc.vector.tensor_copy(lstage[h], pso[DH:DH + 1, :])
                    if h == H - 1:
                        # Last head: finish the PSUM reads before the recip
                        # so the psO bank frees for the out-projection ASAP.
                        nc.vector.tensor_copy(
                            o_sb[h // 2][64:128, :], pso[0:DH, :]
                        )
                    nc.vector.reciprocal_approx_fast(
                        out=linv_st[h], in_=lstage[h]
                    )
                    nc.gpsimd.partition_broadcast(
                        r_sb[h % 2], linv_st[h][0:1, :]
                    )
                    # Even heads: the normalize mul reads O^T straight out
                    # of PSUM (no staging copy — saves ~1.2us DVE per head).
                    # Odd heads need a partition shift 0:64 -> 64:128, which
                    # only plain copies are known to support, so they keep
                    # the staged form.  Split by ic-half so the ic=0
                    # out-proj groups unblock as soon as the first half
                    # lands.
                    if pb == 0:
                        for icm in range(IC):
                            nc.vector.tensor_mul(
                                on_sb[dc][0:64, icm * 512:(icm + 1) * 512],
                                pso[0:DH, icm * 512:(icm + 1) * 512],
                                r_sb[h % 2][0:64, icm * 512:(icm + 1) * 512],
                            )
                    else:
                        if h != H - 1:
                            nc.vector.tensor_copy(
                                o_sb[dc][64:128, :], pso[0:DH, :]
                            )
                        for icm in range(IC):
                            nc.vector.tensor_mul(
                                on_sb[dc][64:128, icm * 512:(icm + 1) * 512],
                                o_sb[dc][64:128, icm * 512:(icm + 1) * 512],
                                r_sb[h % 2][64:128, icm * 512:(icm + 1) * 512],
                            )

                for h, jc in steps:
                    ptile = emit_qk_exp(h, jc)
                    if pending is not None:
                        ph, pjc, pt_ = pending
                        emit_pv(ph, pjc, pt_)
                        if pjc == JC - 1:
                            emit_norm(ph)
                    pending = (h, jc, ptile)
                ph, pjc, pt_ = pending
                emit_pv(ph, pjc, pt_)
                emit_norm(ph)
                # Warm-hold: dependency-free matmuls bridge the PE idle gap
                # between the last attention matmul and the out-projection
                # (whose PSUM banks free only after the norm chain drains
                # through DVE).  Without them HAM sees ~4us of idle and
                # halves the PE clock for the whole out-projection.
                bridge = psS.tile([128, N], f32, tag="psS", name="bridge")
                for _ in range(10):
                    nc.tensor.matmul(
                        bridge[:, 0:512], lhsT=warm_sb[:, 0:128], rhs=warm_sb,
                        start=True, stop=True,
                    )

            # ---- phase E: output projection + bias.  All eight tiles
            # bulk-accumulate hc=0..2 before any hc=3 stop-matmul so the
            # tail after the last head's normalize chain is just 8
            # stop-matmuls + bias + stores.
            with tc.tile_pool(name="pf", bufs=8, space="PSUM") as pf:
                tiles = [pf.tile([128, 512], f32, tag="pf", name=f"pf{q}")
                         for q in range(8)]
                # Per-tile grouped accumulation: a tile whose PSUM bank is
                # still held by the attention pools delays only its own
                # matmuls, not (via the in-order PE queue) everyone else's.
                for q, psf in enumerate(tiles):
                    ec, ic = q // IC, q % IC
                    for hc in range(KC - 1):
                        nc.tensor.matmul(
                            psf,
                            lhsT=wo_sb[hc][:, ec * 128:(ec + 1) * 128],
                            rhs=on_sb[hc][:, ic * 512:(ic + 1) * 512],
                            start=(hc == 0), stop=False,
                        )
                ot_ec = [singles.tile([128, N], bf16, tag=f"ot{e}",
                                      name=f"ot{e}") for e in range(KC)]
                for q, psf in enumerate(tiles):
                    ec, ic = q // IC, q % IC
                    nc.tensor.matmul(
                        psf,
                        lhsT=wo_sb[3][:, ec * 128:(ec + 1) * 128],
                        rhs=on_sb[3][:, ic * 512:(ic + 1) * 512],
                        start=False, stop=True,
                    )
                    # Alternate Act (idle at the tail) and DVE for the bias
                    # adds so neither engine serializes the drain.
                    dst = ot_ec[ec][:, ic * 512:(ic + 1) * 512]
                    if q % 2 == 0:
                        nc.scalar.add(dst, psf, bo_sb[:, ec, :])
                    else:
                        nc.vector.tensor_scalar_add(dst, psf, bo_sb[:, ec, :])
                    if ic == IC - 1:
                        # One [128,1024] store per ec row-block: half the
                        # DMA triggers, on the two HWDGE queues.
                        store_q = (nc.sync, nc.scalar)[ec % 2]
                        store_q.dma_start(
                            out=outT[ec * 128:(ec + 1) * 128, :],
                            in_=ot_ec[ec],
                        )
    # Runs Bacc's compile passes (register allocation, wait splitting) —
    # run_bass_via_pjrt does not finalize for us.
    nc.finalize()
    return nc


def _ensure_ntff_hook():
    """Install antenv.axon_hooks if the image lacks it, registering the
    ctypes NTFF-profile hook against libaxon_pjrt.so. Without this,
    run_bass_kernel_spmd(trace=True)/BASS_TRACE=1 crashes on import."""
    import contextlib
    import ctypes
    import os
    import sys
    import types

    try:
        import antenv.axon_hooks  # noqa: F401
        return
    except ImportError:
        pass
    try:
        import antenv
    except ImportError:
        return

    state = {"hook": None}
    mod = types.ModuleType("antenv.axon_hooks")
    mod.set_axon_ntff_profile_hook = lambda h: state.__setitem__("hook", h)
    mod.get_axon_ntff_profile_hook = lambda: state["hook"]
    sys.modules["antenv.axon_hooks"] = mod
    antenv.axon_hooks = mod

    so_path = "/opt/axon/libaxon_pjrt.so"
    if not os.path.exists(so_path):
        return
    try:
        lib = ctypes.CDLL(so_path)
    except OSError:
        return
    if not hasattr(lib, "axon_start_nrt_profile"):
        return
    lib.axon_start_nrt_profile.argtypes = [
        ctypes.POINTER(ctypes.c_int64), ctypes.c_size_t,
    ]
    lib.axon_start_nrt_profile.restype = ctypes.c_int64
    lib.axon_stop_nrt_profile.argtypes = [ctypes.c_char_p]
    lib.axon_stop_nrt_profile.restype = ctypes.c_int64

    @contextlib.contextmanager
    def _hook(output_dir, device_ids):
        import jax
        jax.devices()  # force PJRT init so the .so's client exists
        if device_ids:
            ids = (ctypes.c_int64 * len(device_ids))(*device_ids)
            rc = lib.axon_start_nrt_profile(ids, len(device_ids))
        else:
            rc = lib.axon_start_nrt_profile(None, 0)
        if rc != 0:
            raise RuntimeError(f"axon_start_nrt_profile rc={rc}")
        try:
            yield
        finally:
            n = lib.axon_stop_nrt_profile(str(output_dir).encode())
            if n <= 0:
                print(f"ntff profile: rc={n} (no profile output)")

    state["hook"] = _hook


def kernel(x, context, Wq, Wkv, Wo, bo):
    global LAST_RUN
    _ensure_ntff_hook()
    from concourse import bass_utils

    if "nc" not in _CACHE:
        _CACHE["nc"] = _build_nc()
    nc = _CACHE["nc"]

    wq = np.ascontiguousarray(Wq, dtype=np.float32).astype(_BF16)
    wk = np.ascontiguousarray(Wkv[:, :D], dtype=np.float32).astype(_BF16)
    wv = np.ascontiguousarray(Wkv[:, D:], dtype=np.float32).astype(_BF16)
    wo = np.ascontiguousarray(Wo, dtype=np.float32).astype(_BF16)
    bo_ = np.ascontiguousarray(np.asarray(bo, dtype=np.float32).reshape(D, 1))
    zb = np.zeros((128, N), dtype=_BF16)

    in_maps = []
    for b in range(B):
        in_maps.append({
            "xt": np.ascontiguousarray(np.asarray(x[b], np.float32).T).astype(_BF16),
            "ct": np.ascontiguousarray(np.asarray(context[b], np.float32).T).astype(_BF16),
            "wq": wq, "wk": wk, "wv": wv, "wo": wo,
            "bo": bo_, "zb": zb,
        })

    LAST_RUN = bass_utils.run_bass_kernel_spmd(nc, in_maps, list(range(N_CORES)))
    out = np.empty((B, N, D), dtype=np.float32)
    for b in range(B):
        out[b] = LAST_RUN.results[b]["outT"].astype(np.float32).T
    return out

